# revision 1
# baseline (speedup 1.0000x reference)
"""Trainium2 Bass kernel for nn_Attention_8495445311883.

Encoder (bi-RNN) + decoder + dot-attention + output projection.
Sharding: data-parallel over batch B=32 across 8 NeuronCores (4 batches/core).
All matmuls in bf16 (fp32 PSUM accumulate). Host pre-packs/transposes weights.

Per-core column index c = b_local*T + t  (b-major), C = 4*T = 512.
Layouts on device (SBUF tiles [128 part, ...]):
  PREF/PREB/PRED [128, 4, 512] f32   : h-tile m on dim1, c on dim2 (h = m*128+p)
  OUTF/OUTB/ENC/ENCT/DEC/CTX [128, 4, 512] bf16
  ENCT[:, b, :] is [t_part, h_free] for batch b; all others are [h_part, c_free].
"""
import os
import sys
import numpy as np

sys.path.insert(0, "/opt/trn_rl_repo")

V, H, T, B = 10000, 512, 128, 32
NCORES = 8
BL = B // NCORES            # 4 local batches
C = BL * T                  # 512 columns per core
VP = 10112                  # V padded to 79*128
KV = VP // 128              # 79 contraction tiles
NV, VC = 20, 500            # output V chunks: 20 x 500

_cached = {}


def _build_nc(reps=1, phases='ASBEXTF', tscan=T):
    import concourse.bacc as bacc
    import concourse.bass as bass
    import concourse.mybir as mybir
    import concourse.tile as tile

    dt = mybir.dt
    AF = mybir.ActivationFunctionType
    AX = mybir.AxisListType

    nc = bacc.Bacc(None, target_bir_lowering=False)

    xT = nc.dram_tensor("xT", [VP, C], dt.bfloat16, kind="ExternalInput")
    dxT = nc.dram_tensor("dxT", [VP, C], dt.bfloat16, kind="ExternalInput")
    WIH = nc.dram_tensor("WIH", [VP, 3 * H], dt.bfloat16, kind="ExternalInput")
    WO = nc.dram_tensor("WO", [2 * H, V], dt.bfloat16, kind="ExternalInput")
    WHH = nc.dram_tensor("WHH", [H, 3 * H], dt.bfloat16, kind="ExternalInput")
    A1 = nc.dram_tensor("A1", [2 * H, H], dt.bfloat16, kind="ExternalInput")
    A2 = nc.dram_tensor("A2", [2 * H, H], dt.bfloat16, kind="ExternalInput")
    CONST = nc.dram_tensor("CONST", [128, 12], dt.float32, kind="ExternalInput")
    BA2 = nc.dram_tensor("BA2", [1, H], dt.bfloat16, kind="ExternalInput")
    BOUT = nc.dram_tensor("BOUT", [1, V], dt.bfloat16, kind="ExternalInput")
    ONES = nc.dram_tensor("ONES", [1, 128], dt.bfloat16, kind="ExternalInput")
    IDN = nc.dram_tensor("IDN", [128, 128], dt.bfloat16, kind="ExternalInput")
    ENCH = nc.dram_tensor("ENCH", [128, 32], dt.bfloat16, kind="ExternalInput")
    out = nc.dram_tensor("out", [BL, T, V], dt.float32, kind="ExternalOutput")

    with tile.TileContext(nc) as tc:
        with (
            tc.tile_pool(name="const", bufs=1) as cp,
            tc.tile_pool(name="acts", bufs=1) as ap,
            tc.tile_pool(name="xs", bufs=3) as xs,
            tc.tile_pool(name="ws", bufs=2) as ws,
            tc.tile_pool(name="os", bufs=4) as osp,
        ):
            # ---- resident constants/weights ----
            whh = cp.tile([128, 4, 3 * H], dt.bfloat16, tag="whh")
            nc.sync.dma_start(whh[:], WHH.rearrange("(j p) c -> p j c", p=128))
            a1 = cp.tile([128, 8, H], dt.bfloat16, tag="a1")
            nc.sync.dma_start(a1[:], A1.rearrange("(j p) c -> p j c", p=128))
            a2 = cp.tile([128, 8, H], dt.bfloat16, tag="a2")
            nc.sync.dma_start(a2[:], A2.rearrange("(j p) c -> p j c", p=128))
            cst = cp.tile([128, 12], dt.float32, tag="cst")
            nc.sync.dma_start(cst[:], CONST[:])
            ba2 = cp.tile([1, H], dt.bfloat16, tag="ba2")
            nc.sync.dma_start(ba2[:], BA2[:])
            bout = cp.tile([1, V], dt.bfloat16, tag="bout")
            nc.sync.dma_start(bout[:], BOUT[:])
            ones = cp.tile([1, 128], dt.bfloat16, tag="ones")
            nc.sync.dma_start(ones[:], ONES[:])
            idn = cp.tile([128, 128], dt.bfloat16, tag="idn")
            nc.sync.dma_start(idn[:], IDN[:])
            ench = cp.tile([128, 32], dt.bfloat16, tag="ench")
            nc.sync.dma_start(ench[:], ENCH[:])

            # ---- persistent activations ----
            PREF = ap.tile([128, 4, C], dt.float32, tag="PREF")
            PREB = ap.tile([128, 4, C], dt.float32, tag="PREB")
            PRED = ap.tile([128, 4, C], dt.float32, tag="PRED")
            OUTF = ap.tile([128, 4, C], dt.bfloat16, tag="OUTF")
            OUTB = ap.tile([128, 4, C], dt.bfloat16, tag="OUTB")
            ENC = ap.tile([128, 4, C], dt.bfloat16, tag="ENC")
            ENCT = ap.tile([128, 4, C], dt.bfloat16, tag="ENCT")
            DEC = ap.tile([128, 4, C], dt.bfloat16, tag="DEC")
            CTX = ap.tile([128, 4, C], dt.bfloat16, tag="CTX")
            H0 = ap.tile([128, 4, 4], dt.bfloat16, tag="H0")
            Q = ap.tile([128, 4, 4], dt.float32, tag="Q")
            TMPF = ap.tile([128, 4, 4], dt.float32, tag="TMPF")
            TMPB = ap.tile([128, 4, 4], dt.float32, tag="TMPB")

            for _rep in range(reps):
                # ---- pass A: encoder input projection (f+b), 8 psum banks ----
                pA_cm = tc.tile_pool(name="pA", bufs=1, space="PSUM"); pA = pA_cm.__enter__()
                psa = [pA.tile([128, C], dt.float32, tag=f"a{m}", name=f"psa{m}") for m in range(8)]
                KG = 4
                NKG = (KV + KG - 1) // KG          # 20 groups, last has 3
                xTr = xT.rearrange("(k p) c -> p k c", p=128)
                WIr = WIH.rearrange("(k p) c -> p k c", p=128)
                for g in (range(NKG) if 'A' in phases else []):
                    ks = list(range(g * KG, min((g + 1) * KG, KV)))
                    xk = xs.tile([128, KG, C], dt.bfloat16, tag="xk")
                    nc.sync.dma_start(xk[:, :len(ks), :], xTr[:, ks[0]:ks[-1] + 1, :])
                    wk = ws.tile([128, KG, 2 * H], dt.bfloat16, tag="wk")
                    nc.scalar.dma_start(wk[:, :len(ks), :],
                                        WIr[:, ks[0]:ks[-1] + 1, 0:2 * H])
                    for i, k in enumerate(ks):
                        for m in range(8):
                            nc.tensor.matmul(
                                psa[m][:], wk[:, i, m * 128:(m + 1) * 128], xk[:, i, :],
                                start=(k == 0), stop=(k == KV - 1),
                            )
                for m in (range(8) if 'A' in phases else []):
                    dst = PREF if m < 4 else PREB
                    nc.vector.tensor_copy(dst[:, m % 4, :], psa[m][:])
                pA_cm.__exit__(None, None, None)

                # ---- pass B (decoder input projection) emitted interleaved with scans ----
                pB_cm = tc.tile_pool(name="pB", bufs=1, space="PSUM"); pB = pB_cm.__enter__()
                psc_cm = tc.tile_pool(name="psc", bufs=1, space="PSUM"); psc = psc_cm.__enter__()
                psb = [pB.tile([128, C], dt.float32, tag=f"b{m}", name=f"psb{m}") for m in range(4)]
                pscf = psc.tile([128, 4, 4], dt.float32, tag="scf")
                pscb = psc.tile([128, 4, 4], dt.float32, tag="scb")

                dxTr = dxT.rearrange("(k p) c -> p k c", p=128)

                def passb_chunk(g):
                    ks = list(range(g * KG, min((g + 1) * KG, KV)))
                    dk = xs.tile([128, KG, C], dt.bfloat16, tag="dk")
                    nc.sync.dma_start(dk[:, :len(ks), :], dxTr[:, ks[0]:ks[-1] + 1, :])
                    wk = ws.tile([128, KG, H], dt.bfloat16, tag="wkd")
                    nc.scalar.dma_start(wk[:, :len(ks), :],
                                        WIr[:, ks[0]:ks[-1] + 1, 2 * H:3 * H])
                    for i, k in enumerate(ks):
                        for m in range(4):
                            nc.tensor.matmul(
                                psb[m][:], wk[:, i, m * 128:(m + 1) * 128], dk[:, i, :],
                                start=(k == 0), stop=(k == KV - 1),
                            )

                kb = 0
                for t in (range(tscan) if 'S' in phases else []):
                    # fwd scan step t ; bwd scan step t (enc-time tb = T-1-t)
                    tb = T - 1 - t
                    for m in range(4):
                        for j in range(4):
                            rf = ench[:, j * 4:(j + 1) * 4] if t == 0 else \
                                OUTF[:, j, (t - 1)::T]
                            nc.tensor.matmul(
                                pscf[:, m, :], whh[:, j, m * 128:(m + 1) * 128], rf,
                                start=(j == 0), stop=(j == 3),
                            )
                        for j in range(4):
                            rb = ench[:, 16 + j * 4:16 + (j + 1) * 4] if t == 0 else \
                                OUTB[:, j, (tb + 1)::T]
                            nc.tensor.matmul(
                                pscb[:, m, :], whh[:, j, H + m * 128:H + (m + 1) * 128], rb,
                                start=(j == 0), stop=(j == 3),
                            )
                    nc.vector.tensor_add(TMPF[:], pscf[:], PREF[:, :, t::T])
                    nc.scalar.activation(OUTF[:, :, t::T], TMPF[:], AF.Tanh)
                    nc.vector.tensor_add(TMPB[:], pscb[:], PREB[:, :, tb::T])
                    nc.scalar.activation(OUTB[:, :, tb::T], TMPB[:], AF.Tanh)
                    # interleave pass-B chunks so PE has filler work
                    while 'B' in phases and kb * T < (t + 1) * NKG:
                        passb_chunk(kb)
                        kb += 1
                while 'B' in phases and kb < NKG:
                    passb_chunk(kb)
                    kb += 1
                for m in (range(4) if 'B' in phases else []):
                    nc.vector.tensor_copy(PRED[:, m, :], psb[m][:])
                psc_cm.__exit__(None, None, None)
                pB_cm.__exit__(None, None, None)
                pmix_cm = tc.tile_pool(name="pmix", bufs=1, space="PSUM"); pmix = pmix_cm.__enter__()

                # ---- ENC = W_attn2 @ [out_f; out_b] + b_attn2   [h_part, c] ----
                for m in (range(4) if 'E' in phases else []):
                    pe = pmix.tile([128, C], dt.float32, tag="pe2")
                    for k in range(8):
                        src = OUTF if k < 4 else OUTB
                        nc.tensor.matmul(
                            pe[:], a2[:, k, m * 128:(m + 1) * 128], src[:, k % 4, :],
                            start=(k == 0), stop=(k == 7),
                        )
                    nc.scalar.activation(ENC[:, m, :], pe[:], AF.Identity,
                                         bias=cst[:, 4 + m:5 + m])

                # ---- ENCT[:, b, :] = [t_part, h] layout of ENC (with bias) ----
                for b in (range(BL) if 'E' in phases else []):
                    pe = pmix.tile([128, H], dt.float32, tag="pet", bufs=2)
                    for k in range(8):
                        src = OUTF if k < 4 else OUTB
                        nc.tensor.matmul(
                            pe[:], src[:, k % 4, b * T:(b + 1) * T], a2[:, k, :],
                            start=(k == 0), stop=False,
                        )
                    nc.tensor.matmul(pe[:], ones[0:1, :], ba2[0:1, :],
                                     start=False, stop=True)
                    nc.scalar.activation(ENCT[:, b, :], pe[:], AF.Copy)

                # ---- h0 = W_attn1 @ [h_f; h_b] + b_attn1 ; q = Whh_d @ h0 + bhh_d ----
                ph = pmix.tile([128, 4, 4], dt.float32, tag="ph0")
                for m in (range(4) if 'X' in phases else []):
                    for k in range(8):
                        rh = OUTF[:, k, (T - 1)::T] if k < 4 else OUTB[:, k - 4, 0::T]
                        nc.tensor.matmul(ph[:, m, :], a1[:, k, m * 128:(m + 1) * 128],
                                         rh, start=(k == 0), stop=(k == 7))
                for m in (range(4) if 'X' in phases else []):
                    nc.scalar.activation(H0[:, m, :], ph[:, m, :], AF.Identity,
                                         bias=cst[:, m:m + 1])
                pq = pmix.tile([128, 4, 4], dt.float32, tag="pq")
                for m in (range(4) if 'X' in phases else []):
                    for j in range(4):
                        nc.tensor.matmul(
                            pq[:, m, :], whh[:, j, 2 * H + m * 128:2 * H + (m + 1) * 128],
                            H0[:, j, :], start=(j == 0), stop=(j == 3),
                        )
                for m in (range(4) if 'X' in phases else []):
                    nc.scalar.activation(Q[:, m, :], pq[:, m, :], AF.Identity,
                                         bias=cst[:, 8 + m:9 + m])

                # ---- DEC = tanh(PRED + q) ----
                for m in (range(4) if 'X' in phases else []):
                    for b in range(BL):
                        nc.scalar.activation(
                            DEC[:, m, b * T:(b + 1) * T], PRED[:, m, b * T:(b + 1) * T],
                            AF.Tanh, bias=Q[:, m, b:b + 1],
                        )

                # ---- attention per batch ----
                for b in (range(BL) if 'T' in phases else []):
                    ps = pmix.tile([128, T], dt.float32, tag="ps")
                    for k in range(4):
                        nc.tensor.matmul(
                            ps[:], DEC[:, k, b * T:(b + 1) * T],
                            ENC[:, k, b * T:(b + 1) * T],
                            start=(k == 0), stop=(k == 3),
                        )
                    negm = osp.tile([128, 1], dt.float32, tag="negm")
                    nc.vector.reduce_max(negm[:], ps[:], axis=AX.X, negate=True)
                    prob = osp.tile([128, T], dt.bfloat16, tag="prob")
                    rsum = osp.tile([128, 1], dt.float32, tag="rsum")
                    nc.scalar.activation(prob[:], ps[:], AF.Exp, bias=negm[:],
                                         accum_out=rsum[:])
                    rinv = osp.tile([128, 1], dt.float32, tag="rinv")
                    nc.vector.reciprocal(rinv[:], rsum[:])
                    nc.vector.tensor_scalar_mul(prob[:], prob[:], rinv[:])
                    pwt = pmix.tile([128, T], dt.bfloat16, tag="pwt")
                    nc.tensor.transpose(pwt[:], prob[:], idn[:])
                    wt = osp.tile([128, T], dt.bfloat16, tag="wt")
                    nc.vector.tensor_copy(wt[:], pwt[:])
                    for m in range(4):
                        pc = pmix.tile([128, T], dt.float32, tag="pc")
                        nc.tensor.matmul(pc[:], ENCT[:, b, m * 128:(m + 1) * 128], wt[:],
                                         start=True, stop=True)
                        nc.scalar.activation(CTX[:, m, b * T:(b + 1) * T], pc[:], AF.Copy)

                pmix_cm.__exit__(None, None, None)
                # ---- final projection: predict[c, v] ----
                pf_cm = tc.tile_pool(name="pf", bufs=4, space="PSUM"); pf = pf_cm.__enter__()
                outr = out.rearrange("b t v -> t b v")
                for n in (range(NV) if 'F' in phases else []):
                    won = ws.tile([128, 8, VC], dt.bfloat16, tag="won")
                    nc.scalar.dma_start(
                        won[:],
                        WO.rearrange("(k p) v -> p k v", p=128)[:, :, n * VC:(n + 1) * VC],
                    )
                    ob = osp.tile([128, BL, VC], dt.float32, tag="ob", bufs=2)
                    for b in range(BL):
                        po = pf.tile([128, VC], dt.float32, tag="po")
                        for k in range(8):
                            src = CTX if k < 4 else DEC
                            nc.tensor.matmul(
                                po[:], src[:, k % 4, b * T:(b + 1) * T], won[:, k, :],
                                start=(k == 0), stop=False,
                            )
                        nc.tensor.matmul(po[:], ones[0:1, :],
                                         bout[0:1, n * VC:(n + 1) * VC],
                                         start=False, stop=True)
                        nc.vector.tensor_copy(ob[:, b, :], po[:])
                    nc.sync.dma_start(outr[:, :, n * VC:(n + 1) * VC], ob[:])
                pf_cm.__exit__(None, None, None)


    nc.compile()
    return nc


def _pack(inputs):
    """Host-side packing: shared weights + per-core activation shards."""
    import ml_dtypes
    bf16 = ml_dtypes.bfloat16
    f = {k: np.asarray(v, dtype=np.float32) for k, v in inputs.items()}

    def bf(a):
        return np.ascontiguousarray(a.astype(bf16))

    WIH = np.zeros((VP, 3 * H), np.float32)
    WIH[:V, 0:H] = f["Wih_f"].T
    WIH[:V, H:2 * H] = f["Wih_b"].T
    WIH[:V, 2 * H:] = f["Wih_d"].T
    WIH[V, 0:H] = f["bih_f"] + f["bhh_f"]
    WIH[V, H:2 * H] = f["bih_b"] + f["bhh_b"]
    WIH[V, 2 * H:] = f["bih_d"]

    WHH = np.concatenate([f["Whh_f"].T, f["Whh_b"].T, f["Whh_d"].T], axis=1)
    CONST = np.concatenate(
        [f["b_attn1"].reshape(4, 128).T, f["b_attn2"].reshape(4, 128).T,
         f["bhh_d"].reshape(4, 128).T], axis=1).astype(np.float32)

    shared = {
        "WIH": bf(WIH),
        "WO": bf(f["W_out"].T),
        "WHH": bf(WHH),
        "A1": bf(f["W_attn1"].T),
        "A2": bf(f["W_attn2"].T),
        "CONST": np.ascontiguousarray(CONST),
        "BA2": bf(f["b_attn2"].reshape(1, H)),
        "BOUT": bf(f["b_out"].reshape(1, V)),
        "ONES": bf(np.ones((1, 128), np.float32)),
        "IDN": bf(np.eye(128, dtype=np.float32)),
    }

    # activations: [V, B*T] with column b*T + t; pad to VP with ones row at V
    def actT(a):  # [B, T, V] -> [VP, B*T]
        r = np.zeros((VP, B * T), np.float32)
        r[:V] = a.transpose(2, 0, 1).reshape(V, B * T)
        r[V] = 1.0
        return bf(r)

    XT = actT(f["enc_inputs"])
    DXT = actT(f["dec_inputs"])

    in_maps = []
    for core in range(NCORES):
        sl = slice(core * C, (core + 1) * C)
        eh = np.zeros((128, 32), np.float32)
        for d in range(2):
            hh = f["enc_hidden"][d, core * BL:(core + 1) * BL]     # [4, 512]
            eh[:, d * 16:(d + 1) * 16] = \
                hh.T.reshape(4, 128, 4).transpose(1, 0, 2).reshape(128, 16)
        m = dict(shared)
        m["xT"] = np.ascontiguousarray(XT[:, sl])
        m["dxT"] = np.ascontiguousarray(DXT[:, sl])
        m["ENCH"] = bf(eh)
        in_maps.append(m)
    return in_maps


def kernel(**inputs):
    from concourse.bass_utils import run_bass_kernel_spmd

    if "nc" not in _cached:
        _cached["nc"] = _build_nc()
    nc = _cached["nc"]
    in_maps = _pack(inputs)
    res = run_bass_kernel_spmd(
        nc, in_maps, core_ids=list(range(NCORES)),
        trace=bool(int(os.environ.get("KTRACE", "0"))),
    )
    _cached["last"] = res
    outp = np.zeros((B, T, V), np.float32)
    for core in range(NCORES):
        outp[core * BL:(core + 1) * BL] = res.results[core]["out"]
    return outp



# revision 2
# speedup vs baseline: 13.1730x; 13.1730x over previous
"""Trainium2 Bass kernel for nn_Attention_8495445311883.

Encoder (bi-RNN) + decoder + dot-attention + output projection.
Sharding: data-parallel over batch B=32 across 8 NeuronCores (4 batches/core).
All matmuls in bf16 (fp32 PSUM accumulate). Host pre-packs/transposes weights.

Per-core column index c = b_local*T + t  (b-major), C = 4*T = 512.
Layouts on device (SBUF tiles [128 part, ...]):
  PREF/PREB/PRED [128, 4, 512] f32   : h-tile m on dim1, c on dim2 (h = m*128+p)
  OUTF/OUTB/ENC/ENCT/DEC/CTX [128, 4, 512] bf16
  ENCT[:, b, :] is [t_part, h_free] for batch b; all others are [h_part, c_free].
"""
import os
import sys
import numpy as np

sys.path.insert(0, "/opt/trn_rl_repo")

V, H, T, B = 10000, 512, 128, 32
NCORES = 8
BL = B // NCORES            # 4 local batches
C = BL * T                  # 512 columns per core
VP = 10112                  # V padded to 79*128
KV = VP // 128              # 79 contraction tiles
NV, VC = 20, 500            # output V chunks: 20 x 500

_cached = {}


def _build_nc(reps=1, phases='ASBEXTF', tscan=T):
    import concourse.bacc as bacc
    import concourse.bass as bass
    import concourse.mybir as mybir
    import concourse.tile as tile

    dt = mybir.dt
    AF = mybir.ActivationFunctionType
    AX = mybir.AxisListType

    nc = bacc.Bacc(None, target_bir_lowering=False)

    xT = nc.dram_tensor("xT", [VP, C], dt.bfloat16, kind="ExternalInput")
    dxT = nc.dram_tensor("dxT", [VP, C], dt.bfloat16, kind="ExternalInput")
    WIH = nc.dram_tensor("WIH", [VP, 3 * H], dt.bfloat16, kind="ExternalInput")
    WO = nc.dram_tensor("WO", [2 * H, V], dt.bfloat16, kind="ExternalInput")
    WHH = nc.dram_tensor("WHH", [H, 3 * H], dt.bfloat16, kind="ExternalInput")
    A1 = nc.dram_tensor("A1", [2 * H, H], dt.bfloat16, kind="ExternalInput")
    A2 = nc.dram_tensor("A2", [2 * H, H], dt.bfloat16, kind="ExternalInput")
    CONST = nc.dram_tensor("CONST", [128, 12], dt.float32, kind="ExternalInput")
    BA2 = nc.dram_tensor("BA2", [1, H], dt.bfloat16, kind="ExternalInput")
    BOUT = nc.dram_tensor("BOUT", [1, V], dt.bfloat16, kind="ExternalInput")
    ONES = nc.dram_tensor("ONES", [1, 128], dt.bfloat16, kind="ExternalInput")
    IDN = nc.dram_tensor("IDN", [128, 128], dt.bfloat16, kind="ExternalInput")
    ENCH = nc.dram_tensor("ENCH", [128, 32], dt.bfloat16, kind="ExternalInput")
    out = nc.dram_tensor("out", [BL, T, V], dt.float32, kind="ExternalOutput")

    with tile.TileContext(nc) as tc:
        with (
            tc.tile_pool(name="const", bufs=1) as cp,
            tc.tile_pool(name="acts", bufs=1) as ap,
            tc.tile_pool(name="xs", bufs=3) as xs,
            tc.tile_pool(name="ws", bufs=2) as ws,
            tc.tile_pool(name="os", bufs=4) as osp,
        ):
            # ---- resident constants/weights ----
            whh = cp.tile([128, 4, 3 * H], dt.bfloat16, tag="whh")
            nc.sync.dma_start(whh[:], WHH.rearrange("(j p) c -> p j c", p=128))
            a1 = cp.tile([128, 8, H], dt.bfloat16, tag="a1")
            nc.sync.dma_start(a1[:], A1.rearrange("(j p) c -> p j c", p=128))
            a2 = cp.tile([128, 8, H], dt.bfloat16, tag="a2")
            nc.sync.dma_start(a2[:], A2.rearrange("(j p) c -> p j c", p=128))
            cst = cp.tile([128, 12], dt.float32, tag="cst")
            nc.sync.dma_start(cst[:], CONST[:])
            ba2 = cp.tile([1, H], dt.bfloat16, tag="ba2")
            nc.sync.dma_start(ba2[:], BA2[:])
            bout = cp.tile([1, V], dt.bfloat16, tag="bout")
            nc.sync.dma_start(bout[:], BOUT[:])
            ones = cp.tile([1, 128], dt.bfloat16, tag="ones")
            nc.sync.dma_start(ones[:], ONES[:])
            idn = cp.tile([128, 128], dt.bfloat16, tag="idn")
            nc.sync.dma_start(idn[:], IDN[:])
            ench = cp.tile([128, 32], dt.bfloat16, tag="ench")
            nc.sync.dma_start(ench[:], ENCH[:])

            # ---- persistent activations ----
            PREF = ap.tile([128, 4, C], dt.float32, tag="PREF")
            PREB = ap.tile([128, 4, C], dt.float32, tag="PREB")
            PRED = ap.tile([128, 4, C], dt.float32, tag="PRED")
            OUTF = ap.tile([128, 4, C], dt.bfloat16, tag="OUTF")
            OUTB = ap.tile([128, 4, C], dt.bfloat16, tag="OUTB")
            ENC = ap.tile([128, 4, C], dt.bfloat16, tag="ENC")
            ENCT = ap.tile([128, 4, C], dt.bfloat16, tag="ENCT")
            DEC = ap.tile([128, 4, C], dt.bfloat16, tag="DEC")
            CTX = ap.tile([128, 4, C], dt.bfloat16, tag="CTX")
            H0 = ap.tile([128, 4, 4], dt.bfloat16, tag="H0")
            Q = ap.tile([128, 4, 4], dt.float32, tag="Q")
            TMPF = ap.tile([128, 4, 4], dt.float32, tag="TMPF")
            TMPB = ap.tile([128, 4, 4], dt.float32, tag="TMPB")

            for _rep in range(reps):
                # ---- pass A: encoder input projection (f+b), 8 psum banks ----
                pA_cm = tc.tile_pool(name="pA", bufs=1, space="PSUM"); pA = pA_cm.__enter__()
                psa = [pA.tile([128, C], dt.float32, tag=f"a{m}", name=f"psa{m}") for m in range(8)]
                KG = 4
                NKG = (KV + KG - 1) // KG          # 20 groups, last has 3
                xTr = xT.rearrange("(k p) c -> p k c", p=128)
                WIr = WIH.rearrange("(k p) c -> p k c", p=128)
                for g in (range(NKG) if 'A' in phases else []):
                    ks = list(range(g * KG, min((g + 1) * KG, KV)))
                    xk = xs.tile([128, KG, C], dt.bfloat16, tag="xk")
                    nc.sync.dma_start(xk[:, :len(ks), :], xTr[:, ks[0]:ks[-1] + 1, :])
                    wk = ws.tile([128, KG, 2 * H], dt.bfloat16, tag="wk")
                    nc.scalar.dma_start(wk[:, :len(ks), :],
                                        WIr[:, ks[0]:ks[-1] + 1, 0:2 * H])
                    for i, k in enumerate(ks):
                        for m in range(8):
                            nc.tensor.matmul(
                                psa[m][:], wk[:, i, m * 128:(m + 1) * 128], xk[:, i, :],
                                start=(k == 0), stop=(k == KV - 1),
                            )
                for m in (range(8) if 'A' in phases else []):
                    dst = PREF if m < 4 else PREB
                    nc.vector.tensor_copy(dst[:, m % 4, :], psa[m][:])
                pA_cm.__exit__(None, None, None)

                # ---- pass B (decoder input projection) emitted interleaved with scans ----
                pB_cm = tc.tile_pool(name="pB", bufs=1, space="PSUM"); pB = pB_cm.__enter__()
                psc_cm = tc.tile_pool(name="psc", bufs=1, space="PSUM"); psc = psc_cm.__enter__()
                psb = [pB.tile([128, C], dt.float32, tag=f"b{m}", name=f"psb{m}") for m in range(4)]
                pscf = psc.tile([128, 4, 4], dt.float32, tag="scf")
                pscb = psc.tile([128, 4, 4], dt.float32, tag="scb")

                dxTr = dxT.rearrange("(k p) c -> p k c", p=128)

                def passb_chunk(g):
                    ks = list(range(g * KG, min((g + 1) * KG, KV)))
                    dk = xs.tile([128, KG, C], dt.bfloat16, tag="dk")
                    nc.sync.dma_start(dk[:, :len(ks), :], dxTr[:, ks[0]:ks[-1] + 1, :])
                    wk = ws.tile([128, KG, H], dt.bfloat16, tag="wkd")
                    nc.scalar.dma_start(wk[:, :len(ks), :],
                                        WIr[:, ks[0]:ks[-1] + 1, 2 * H:3 * H])
                    for i, k in enumerate(ks):
                        for m in range(4):
                            nc.tensor.matmul(
                                psb[m][:], wk[:, i, m * 128:(m + 1) * 128], dk[:, i, :],
                                start=(k == 0), stop=(k == KV - 1),
                            )

                kb = 0
                for t in (range(tscan) if 'S' in phases else []):
                    # fwd scan step t ; bwd scan step t (enc-time tb = T-1-t)
                    tb = T - 1 - t
                    for m in range(4):
                        for j in range(4):
                            rf = ench[:, j * 4:(j + 1) * 4] if t == 0 else \
                                OUTF[:, j, (t - 1)::T]
                            nc.tensor.matmul(
                                pscf[:, m, :], whh[:, j, m * 128:(m + 1) * 128], rf,
                                start=(j == 0), stop=(j == 3),
                            )
                        for j in range(4):
                            rb = ench[:, 16 + j * 4:16 + (j + 1) * 4] if t == 0 else \
                                OUTB[:, j, (tb + 1)::T]
                            nc.tensor.matmul(
                                pscb[:, m, :], whh[:, j, H + m * 128:H + (m + 1) * 128], rb,
                                start=(j == 0), stop=(j == 3),
                            )
                    nc.vector.tensor_add(TMPF[:], pscf[:], PREF[:, :, t::T])
                    nc.scalar.activation(OUTF[:, :, t::T], TMPF[:], AF.Tanh)
                    nc.vector.tensor_add(TMPB[:], pscb[:], PREB[:, :, tb::T])
                    nc.scalar.activation(OUTB[:, :, tb::T], TMPB[:], AF.Tanh)
                    # interleave pass-B chunks so PE has filler work
                    while 'B' in phases and kb * T < (t + 1) * NKG:
                        passb_chunk(kb)
                        kb += 1
                while 'B' in phases and kb < NKG:
                    passb_chunk(kb)
                    kb += 1
                for m in (range(4) if 'B' in phases else []):
                    nc.vector.tensor_copy(PRED[:, m, :], psb[m][:])
                psc_cm.__exit__(None, None, None)
                pB_cm.__exit__(None, None, None)
                pmix_cm = tc.tile_pool(name="pmix", bufs=1, space="PSUM"); pmix = pmix_cm.__enter__()

                # ---- ENC = W_attn2 @ [out_f; out_b] + b_attn2   [h_part, c] ----
                for m in (range(4) if 'E' in phases else []):
                    pe = pmix.tile([128, C], dt.float32, tag="pe2")
                    for k in range(8):
                        src = OUTF if k < 4 else OUTB
                        nc.tensor.matmul(
                            pe[:], a2[:, k, m * 128:(m + 1) * 128], src[:, k % 4, :],
                            start=(k == 0), stop=(k == 7),
                        )
                    nc.scalar.activation(ENC[:, m, :], pe[:], AF.Identity,
                                         bias=cst[:, 4 + m:5 + m])

                # ---- ENCT[:, b, :] = [t_part, h] layout of ENC (with bias) ----
                for b in (range(BL) if 'E' in phases else []):
                    pe = pmix.tile([128, H], dt.float32, tag="pet", bufs=2)
                    for k in range(8):
                        src = OUTF if k < 4 else OUTB
                        nc.tensor.matmul(
                            pe[:], src[:, k % 4, b * T:(b + 1) * T], a2[:, k, :],
                            start=(k == 0), stop=False,
                        )
                    nc.tensor.matmul(pe[:], ones[0:1, :], ba2[0:1, :],
                                     start=False, stop=True)
                    nc.scalar.activation(ENCT[:, b, :], pe[:], AF.Copy)

                # ---- h0 = W_attn1 @ [h_f; h_b] + b_attn1 ; q = Whh_d @ h0 + bhh_d ----
                ph = pmix.tile([128, 4, 4], dt.float32, tag="ph0")
                for m in (range(4) if 'X' in phases else []):
                    for k in range(8):
                        rh = OUTF[:, k, (T - 1)::T] if k < 4 else OUTB[:, k - 4, 0::T]
                        nc.tensor.matmul(ph[:, m, :], a1[:, k, m * 128:(m + 1) * 128],
                                         rh, start=(k == 0), stop=(k == 7))
                for m in (range(4) if 'X' in phases else []):
                    nc.scalar.activation(H0[:, m, :], ph[:, m, :], AF.Identity,
                                         bias=cst[:, m:m + 1])
                pq = pmix.tile([128, 4, 4], dt.float32, tag="pq")
                for m in (range(4) if 'X' in phases else []):
                    for j in range(4):
                        nc.tensor.matmul(
                            pq[:, m, :], whh[:, j, 2 * H + m * 128:2 * H + (m + 1) * 128],
                            H0[:, j, :], start=(j == 0), stop=(j == 3),
                        )
                for m in (range(4) if 'X' in phases else []):
                    nc.scalar.activation(Q[:, m, :], pq[:, m, :], AF.Identity,
                                         bias=cst[:, 8 + m:9 + m])

                # ---- DEC = tanh(PRED + q) ----
                for m in (range(4) if 'X' in phases else []):
                    for b in range(BL):
                        nc.scalar.activation(
                            DEC[:, m, b * T:(b + 1) * T], PRED[:, m, b * T:(b + 1) * T],
                            AF.Tanh, bias=Q[:, m, b:b + 1],
                        )

                # ---- attention per batch ----
                for b in (range(BL) if 'T' in phases else []):
                    ps = pmix.tile([128, T], dt.float32, tag="ps")
                    for k in range(4):
                        nc.tensor.matmul(
                            ps[:], DEC[:, k, b * T:(b + 1) * T],
                            ENC[:, k, b * T:(b + 1) * T],
                            start=(k == 0), stop=(k == 3),
                        )
                    negm = osp.tile([128, 1], dt.float32, tag="negm")
                    nc.vector.reduce_max(negm[:], ps[:], axis=AX.X, negate=True)
                    prob = osp.tile([128, T], dt.bfloat16, tag="prob")
                    rsum = osp.tile([128, 1], dt.float32, tag="rsum")
                    nc.scalar.activation(prob[:], ps[:], AF.Exp, bias=negm[:],
                                         accum_out=rsum[:])
                    rinv = osp.tile([128, 1], dt.float32, tag="rinv")
                    nc.vector.reciprocal(rinv[:], rsum[:])
                    nc.vector.tensor_scalar_mul(prob[:], prob[:], rinv[:])
                    pwt = pmix.tile([128, T], dt.bfloat16, tag="pwt")
                    nc.tensor.transpose(pwt[:], prob[:], idn[:])
                    wt = osp.tile([128, T], dt.bfloat16, tag="wt")
                    nc.vector.tensor_copy(wt[:], pwt[:])
                    for m in range(4):
                        pc = pmix.tile([128, T], dt.float32, tag="pc")
                        nc.tensor.matmul(pc[:], ENCT[:, b, m * 128:(m + 1) * 128], wt[:],
                                         start=True, stop=True)
                        nc.scalar.activation(CTX[:, m, b * T:(b + 1) * T], pc[:], AF.Copy)

                pmix_cm.__exit__(None, None, None)
                # ---- final projection: predict[c, v] ----
                pf_cm = tc.tile_pool(name="pf", bufs=4, space="PSUM"); pf = pf_cm.__enter__()
                outr = out.rearrange("b t v -> t b v")
                for n in (range(NV) if 'F' in phases else []):
                    won = ws.tile([128, 8, VC], dt.bfloat16, tag="won")
                    nc.scalar.dma_start(
                        won[:],
                        WO.rearrange("(k p) v -> p k v", p=128)[:, :, n * VC:(n + 1) * VC],
                    )
                    ob = osp.tile([128, BL, VC], dt.float32, tag="ob", bufs=2)
                    for b in range(BL):
                        po = pf.tile([128, VC], dt.float32, tag="po")
                        for k in range(8):
                            src = CTX if k < 4 else DEC
                            nc.tensor.matmul(
                                po[:], src[:, k % 4, b * T:(b + 1) * T], won[:, k, :],
                                start=(k == 0), stop=False,
                            )
                        nc.tensor.matmul(po[:], ones[0:1, :],
                                         bout[0:1, n * VC:(n + 1) * VC],
                                         start=False, stop=True)
                        nc.vector.tensor_copy(ob[:, b, :], po[:])
                    nc.sync.dma_start(outr[:, :, n * VC:(n + 1) * VC], ob[:])
                pf_cm.__exit__(None, None, None)


    nc.compile()
    return nc


def _pack(inputs):
    """Host-side packing: shared weights + per-core activation shards."""
    import ml_dtypes
    bf16 = ml_dtypes.bfloat16
    f = {k: np.asarray(v, dtype=np.float32) for k, v in inputs.items()}

    def bf(a):
        return np.ascontiguousarray(a.astype(bf16))

    WIH = np.zeros((VP, 3 * H), np.float32)
    WIH[:V, 0:H] = f["Wih_f"].T
    WIH[:V, H:2 * H] = f["Wih_b"].T
    WIH[:V, 2 * H:] = f["Wih_d"].T
    WIH[V, 0:H] = f["bih_f"] + f["bhh_f"]
    WIH[V, H:2 * H] = f["bih_b"] + f["bhh_b"]
    WIH[V, 2 * H:] = f["bih_d"]

    WHH = np.concatenate([f["Whh_f"].T, f["Whh_b"].T, f["Whh_d"].T], axis=1)
    CONST = np.concatenate(
        [f["b_attn1"].reshape(4, 128).T, f["b_attn2"].reshape(4, 128).T,
         f["bhh_d"].reshape(4, 128).T], axis=1).astype(np.float32)

    shared = {
        "WIH": bf(WIH),
        "WO": bf(f["W_out"].T),
        "WHH": bf(WHH),
        "A1": bf(f["W_attn1"].T),
        "A2": bf(f["W_attn2"].T),
        "CONST": np.ascontiguousarray(CONST),
        "BA2": bf(f["b_attn2"].reshape(1, H)),
        "BOUT": bf(f["b_out"].reshape(1, V)),
        "ONES": bf(np.ones((1, 128), np.float32)),
        "IDN": bf(np.eye(128, dtype=np.float32)),
    }

    # activations: [V, B*T] with column b*T + t; pad to VP with ones row at V
    def actT(a):  # [B, T, V] -> [VP, B*T]
        r = np.zeros((VP, B * T), np.float32)
        r[:V] = a.transpose(2, 0, 1).reshape(V, B * T)
        r[V] = 1.0
        return bf(r)

    XT = actT(f["enc_inputs"])
    DXT = actT(f["dec_inputs"])

    in_maps = []
    for core in range(NCORES):
        sl = slice(core * C, (core + 1) * C)
        eh = np.zeros((128, 32), np.float32)
        for d in range(2):
            hh = f["enc_hidden"][d, core * BL:(core + 1) * BL]     # [4, 512]
            eh[:, d * 16:(d + 1) * 16] = \
                hh.T.reshape(4, 128, 4).transpose(1, 0, 2).reshape(128, 16)
        m = dict(shared)
        m["xT"] = np.ascontiguousarray(XT[:, sl])
        m["dxT"] = np.ascontiguousarray(DXT[:, sl])
        m["ENCH"] = bf(eh)
        in_maps.append(m)
    return in_maps


def _unpack_core(res, core):
    """res: dict of this core's output tensors -> [BL, T, V] float32."""
    return np.asarray(res["out"], dtype=np.float32)


def kernel(**inputs):
    from concourse.bass_utils import run_bass_kernel_spmd

    if "nc" not in _cached:
        _cached["nc"] = _build_nc()
    nc = _cached["nc"]
    in_maps = _pack(inputs)
    res = run_bass_kernel_spmd(
        nc, in_maps, core_ids=list(range(NCORES)),
        trace=bool(int(os.environ.get("KTRACE", "0"))),
    )
    _cached["last"] = res
    outp = np.zeros((B, T, V), np.float32)
    for core in range(NCORES):
        outp[core * BL:(core + 1) * BL] = _unpack_core(res.results[core], core)
    return outp



# revision 28
# speedup vs baseline: 2470.8828x; 187.5715x over previous
"""Trainium2 Bass kernel for nn_Attention_8495445311883.

Encoder (bi-RNN) + decoder + dot-attention + output projection.
Sharding: data-parallel over batch B=32 across 8 NeuronCores (4 batches/core).
Per-core column index c = b_local*T + t  (b-major), C = 4*T = 512.

The three big GEMMs (encoder/decoder input projections, output projection)
run in fp8-e4m3 DoubleRow mode (0.5 cycles/row) with 3-term error
compensation: each operand is split hi+lo (lo = fp8 residual), and
W@X ~= Wh@Xh + Wh@Xl + Wl@Xh accumulated in fp32 PSUM -- 0.75x the PE time
of bf16 at ~bf16 accuracy. Weights are pre-scaled x64 on the host so their
lo residuals clear fp8's denormal floor; the 1/64 is folded into the
PSUM->SBUF copies/activations. DMA-lean: x, dx, WIH (hi+lo fp8 = bf16
bytes), and W_out are each streamed exactly once.

Schedule: pass A (one full-column k-sweep, 8 PSUM banks) -> fwd/bwd RNN
scans with pass B and progressive-ENC as fillers -> per-batch softmax
pipeline -> v-partitioned output projection. The scan's per-step chain is
kept short: DVE preloads pre into PSUM two steps at a time (4-slot ring),
PE accumulates Whh@h on top (start=False), one merged Activation tanh per
step covers both directions. Bwd state is produced in scan order (OUTS dir
1) and un-reversed into OUTB by a GPSIMD copy per step; scan steps >= 64
also compute ENC columns t=s and t=127-s (both directions complete there),
pulling most post-scan work into the otherwise chain-bound scan window.

Final projection is v-partitioned: out[v, c] = W_out.T[2H, v] contracted
against [CTX; DEC] (converted to fp8 hi/lo on-chip), bias added
per-partition during PSUM->SBUF, stored bf16 as [VPO, C] DRAM and
transposed on the host. GPSIMD takes SBUF-to-SBUF copies only (it cannot
access PSUM on real HW).
"""
import os
import sys
import numpy as np

sys.path.insert(0, "/opt/trn_rl_repo")

V, H, T, B = 10000, 512, 128, 32
NCORES = 8
BL = B // NCORES            # 4 local batches
C = BL * T                  # 512 columns per core
VP = 10240                  # V padded to 80*128 (input contraction)
KV = VP // 128              # 80 contraction tiles (40 DoubleRow pairs)
VPO = 10240                 # V padded to 80*128 (output rows)
VT2 = VPO // 256            # 40 output super-tiles (2 v-tiles each)

_cached = {}


def _build_nc(reps=1):
    import concourse.bacc as bacc
    import concourse.bass as bass
    import concourse.mybir as mybir
    import concourse.tile as tile

    dt = mybir.dt
    AF = mybir.ActivationFunctionType
    AX = mybir.AxisListType

    nc = bacc.Bacc(None, target_bir_lowering=False)

    F8 = dt.float8e4
    DR = mybir.MatmulPerfMode.DoubleRow
    xTH = nc.dram_tensor("xTH", [VP, C], F8, kind="ExternalInput")
    xTL = nc.dram_tensor("xTL", [VP, C], F8, kind="ExternalInput")
    dxTH = nc.dram_tensor("dxTH", [VP, C], F8, kind="ExternalInput")
    dxTL = nc.dram_tensor("dxTL", [VP, C], F8, kind="ExternalInput")
    WIHH = nc.dram_tensor("WIHH", [VP, 3 * H], F8, kind="ExternalInput")
    WIHL = nc.dram_tensor("WIHL", [VP, 3 * H], F8, kind="ExternalInput")
    WOT8 = nc.dram_tensor("WOT8", [VT2, 2 * H, 512], F8, kind="ExternalInput")
    WHH = nc.dram_tensor("WHH", [H, 3 * H], dt.bfloat16, kind="ExternalInput")
    A1 = nc.dram_tensor("A1", [2 * H, H], dt.bfloat16, kind="ExternalInput")
    A2 = nc.dram_tensor("A2", [2 * H, H], dt.bfloat16, kind="ExternalInput")
    CONST = nc.dram_tensor("CONST", [128, 12], dt.float32, kind="ExternalInput")
    BOUTP = nc.dram_tensor("BOUTP", [128, 2 * VT2], dt.float32, kind="ExternalInput")
    BA2 = nc.dram_tensor("BA2", [1, H], dt.bfloat16, kind="ExternalInput")
    ONES = nc.dram_tensor("ONES", [1, 128], dt.bfloat16, kind="ExternalInput")
    IDN = nc.dram_tensor("IDN", [128, 128], dt.bfloat16, kind="ExternalInput")
    ENCH = nc.dram_tensor("ENCH", [128, 32], dt.bfloat16, kind="ExternalInput")
    out = nc.dram_tensor("out", [VPO, C], dt.bfloat16, kind="ExternalOutput")

    xTHr = xTH.rearrange("(k p) c -> p k c", p=128)
    xTLr = xTL.rearrange("(k p) c -> p k c", p=128)
    dxTHr = dxTH.rearrange("(k p) c -> p k c", p=128)
    dxTLr = dxTL.rearrange("(k p) c -> p k c", p=128)
    WIHr = WIHH.rearrange("(k p) c -> p k c", p=128)
    WILr = WIHL.rearrange("(k p) c -> p k c", p=128)
    outr = out.rearrange("(w p) c -> p w c", p=128)

    KG = 4                              # k-tiles per DMA chunk
    NKG = (KV + KG - 1) // KG           # 20 chunks (last has 3)

    with tile.TileContext(nc) as tc:
        with (
            tc.tile_pool(name="const", bufs=1) as cp,
            tc.tile_pool(name="acts", bufs=1) as ap,
            tc.tile_pool(name="xs", bufs=5) as xs,
            tc.tile_pool(name="ws", bufs=2) as ws,
            tc.tile_pool(name="os", bufs=4) as osp,
        ):
            # ---- resident constants/weights (DMAs emitted after sweep1's
            # first chunks so the PE isn't gated on them at startup) ----
            whh = cp.tile([128, 4, 3 * H], dt.bfloat16, tag="whh")
            a1 = cp.tile([128, 8, H], dt.bfloat16, tag="a1")
            a2 = cp.tile([128, 8, H], dt.bfloat16, tag="a2")
            cst = cp.tile([128, 12], dt.float32, tag="cst")
            boutp = cp.tile([128, 2 * VT2], dt.float32, tag="boutp")
            ba2 = cp.tile([1, H], dt.bfloat16, tag="ba2")
            ones = cp.tile([1, 128], dt.bfloat16, tag="ones")
            idn = cp.tile([128, 128], dt.bfloat16, tag="idn")
            ench = cp.tile([128, 32], dt.bfloat16, tag="ench")

            def const_dmas():
                nc.sync.dma_start(whh[:], WHH.rearrange("(j p) c -> p j c", p=128))
                nc.sync.dma_start(a1[:], A1.rearrange("(j p) c -> p j c", p=128))
                nc.sync.dma_start(a2[:], A2.rearrange("(j p) c -> p j c", p=128))
                nc.sync.dma_start(cst[:], CONST[:])
                nc.sync.dma_start(boutp[:], BOUTP[:])
                nc.sync.dma_start(ba2[:], BA2[:])
                nc.sync.dma_start(ones[:], ONES[:])
                nc.sync.dma_start(idn[:], IDN[:])
                nc.sync.dma_start(ench[:], ENCH[:])

            # ---- persistent activations ----
            # PREFB: dir 0 = fwd pre (enc order), dir 1 = bwd pre (scan order)
            PREFB = ap.tile([128, 2, 4, C], dt.float32, tag="PREFB")
            PRED = ap.tile([128, 4, C], dt.float32, tag="PRED")
            # OUTS: dir 0 = fwd h (enc order == scan order), dir 1 = bwd h
            # (scan order); OUTB = bwd h in enc order (un-reversed per step)
            OUTS = ap.tile([128, 2, 4, C], dt.bfloat16, tag="OUTS")
            OUTB = ap.tile([128, 4, C], dt.bfloat16, tag="OUTB")
            ENC = ap.tile([128, 4, C], dt.bfloat16, tag="ENC")
            ENCT = ap.tile([128, 4, C], dt.bfloat16, tag="ENCT")
            DEC = ap.tile([128, 4, C], dt.bfloat16, tag="DEC")
            CTX = ap.tile([128, 4, C], dt.bfloat16, tag="CTX")
            H0 = ap.tile([128, 4, 4], dt.bfloat16, tag="H0")
            Q = ap.tile([128, 4, 4], dt.float32, tag="Q")
            CAT8H = ap.tile([128, 8, C], F8, tag="CAT8H")
            CAT8L = ap.tile([128, 8, C], F8, tag="CAT8L")

            for _rep in range(reps):
                # ================= pass A (full columns, one k-sweep) ======
                pA_cm = tc.tile_pool(name="pA", bufs=1, space="PSUM")
                pA = pA_cm.__enter__()
                psa = [pA.tile([128, C], dt.float32, tag=f"a{m}",
                               name=f"psa{m}") for m in range(8)]

                def sweep_chunk(g):
                    k0 = g * KG
                    xkh = xs.tile([128, KG, C], F8, tag="xkh")
                    nc.sync.dma_start(xkh[:], xTHr[:, k0:k0 + KG, :])
                    xkl = xs.tile([128, KG, C], F8, tag="xkl")
                    nc.sync.dma_start(xkl[:], xTLr[:, k0:k0 + KG, :])
                    wkh = ws.tile([128, KG, 2 * H], F8, tag="wkh")
                    nc.scalar.dma_start(wkh[:], WIHr[:, k0:k0 + KG, 0:2 * H])
                    wkl = ws.tile([128, KG, 2 * H], F8, tag="wkl")
                    nc.scalar.dma_start(wkl[:], WILr[:, k0:k0 + KG, 0:2 * H])
                    for i in range(0, KG, 2):
                        pg = g * (KG // 2) + i // 2
                        first, last = pg == 0, pg == KV // 2 - 1
                        for m in range(8):
                            ms = slice(m * 128, (m + 1) * 128)
                            terms = [(wkh, xkh, first, False),
                                     (wkh, xkl, False, False),
                                     (wkl, xkh, False, last)]
                            for wt_, xt_, st, sp in terms:
                                nc.tensor.matmul(
                                    psa[m][:], wt_[:, i:i + 2, ms],
                                    xt_[:, i:i + 2, :],
                                    start=st, stop=sp, perf_mode=DR,
                                )

                def sweep_copyout():
                    # fwd in enc order; bwd reversed into scan order
                    # (1/64 undoes the x64 host scaling of WIH for fp8)
                    pv = PREFB.rearrange("p d m (b t) -> p d m b t", b=4)
                    for m in range(4):
                        nc.vector.tensor_scalar_mul(PREFB[:, 0, m, :],
                                                    psa[m][:], 1.0 / 64.0)
                    for m in range(4):
                        pb = psa[4 + m].rearrange("p (b t) -> p b t", b=4)
                        nc.vector.tensor_scalar_mul(pv[:, 1, m, :, :],
                                                    pb[:, :, ::-1], 1.0 / 64.0)

                # pass B chunk (decoder input projection), 4 full psum banks
                def passb_chunk(pB, g):
                    k0 = g * KG
                    dkh = xs.tile([128, KG, C], F8, tag="dkh")
                    nc.sync.dma_start(dkh[:], dxTHr[:, k0:k0 + KG, :])
                    dkl = xs.tile([128, KG, C], F8, tag="dkl")
                    nc.sync.dma_start(dkl[:], dxTLr[:, k0:k0 + KG, :])
                    wdh = ws.tile([128, KG, H], F8, tag="wdh")
                    nc.scalar.dma_start(wdh[:], WIHr[:, k0:k0 + KG, 2 * H:3 * H])
                    wdl = ws.tile([128, KG, H], F8, tag="wdl")
                    nc.scalar.dma_start(wdl[:], WILr[:, k0:k0 + KG, 2 * H:3 * H])
                    for i in range(0, KG, 2):
                        pg = g * (KG // 2) + i // 2
                        first, last = pg == 0, pg == KV // 2 - 1
                        for m in range(4):
                            ms = slice(m * 128, (m + 1) * 128)
                            terms = [(wdh, dkh, first, False),
                                     (wdh, dkl, False, False),
                                     (wdl, dkh, False, last)]
                            for wt_, xt_, st, sp in terms:
                                nc.tensor.matmul(
                                    pB[m][:], wt_[:, i:i + 2, ms],
                                    xt_[:, i:i + 2, :],
                                    start=st, stop=sp, perf_mode=DR,
                                )

                def scan_step(s):
                    par = PSC[:, s % 4]
                    # preload pre for steps (s, s+1) into PSUM (DVE)
                    if s % 2 == 0:
                        nc.vector.tensor_copy(PSC[:, s % 4:s % 4 + 2],
                                              PREV[:, s:s + 2])
                    for m in range(4):
                        for j in range(4):
                            rf = ench[:, j * 4:(j + 1) * 4] if s == 0 else \
                                OUTS[:, 0, j, (s - 1)::T]
                            nc.tensor.matmul(
                                par[:, 0, m, :], whh[:, j, m * 128:(m + 1) * 128],
                                rf, start=False, stop=(j == 3),
                                skip_group_check=True,
                            )
                        for j in range(4):
                            rb = ench[:, 16 + j * 4:16 + (j + 1) * 4] if s == 0 \
                                else OUTS[:, 1, j, (s - 1)::T]
                            nc.tensor.matmul(
                                par[:, 1, m, :],
                                whh[:, j, H + m * 128:H + (m + 1) * 128],
                                rb, start=False, stop=(j == 3),
                                skip_group_check=True,
                            )
                    nc.scalar.activation(OUTS[:, :, :, s::T], par[:], AF.Tanh)
                    # un-reverse bwd state into enc order (off critical path)
                    nc.gpsimd.tensor_copy(OUTB[:, :, (T - 1 - s)::T],
                                          OUTS[:, 1, :, s::T])
                    # progressive ENC: cols t=s and t=T-1-s are complete now
                    if s >= T // 2:
                        pp = PSC2[:, s % 2]
                        for tt, t in enumerate((s, T - 1 - s)):
                            for m in range(4):
                                ms = slice(m * 128, (m + 1) * 128)
                                for k in range(8):
                                    mv = OUTS[:, 0, k, t::T] if k < 4 else \
                                        OUTB[:, k - 4, t::T]
                                    nc.tensor.matmul(
                                        pp[:, tt, m, :], a2[:, k, ms], mv,
                                        start=(tt == 0 and m == 0 and k == 0),
                                        stop=False, skip_group_check=True)
                                nc.tensor.matmul(
                                    pp[:, tt, m, :], ba2[0:1, ms],
                                    ones[0:1, 0:4], start=False,
                                    stop=(tt == 1 and m == 3),
                                    skip_group_check=True)
                        nc.vector.tensor_copy(ENC[:, :, s::T], pp[:, 0])
                        nc.vector.tensor_copy(ENC[:, :, (T - 1 - s)::T],
                                              pp[:, 1])

                # --- pass A solid, then scan with pass B interleaved ---
                for g in range(NKG):
                    sweep_chunk(g)
                    if g == 1 and _rep == 0:
                        const_dmas()
                sweep_copyout()
                pA_cm.__exit__(None, None, None)

                psc_cm = tc.tile_pool(name="psc", bufs=1, space="PSUM")
                psc = psc_cm.__enter__()
                # scan psum: one bank, 4 step-buffers via dim 1
                PSC = psc.tile([128, 4, 2, 4, 4], dt.float32, tag="sc",
                               name="PSC")
                # pre in (p, t, dir, m, b) order for paired preloads
                PREV = PREFB.rearrange("p d m (b t) -> p t d m b", b=4)
                # progressive-ENC psum: [buf, t-slot, m, b]
                PSC2 = psc.tile([128, 2, 2, 4, 4], dt.float32, tag="sc2",
                                name="PSC2")
                pB_cm = tc.tile_pool(name="pB", bufs=1, space="PSUM")
                pBp = pB_cm.__enter__()
                psb = [pBp.tile([128, C], dt.float32, tag=f"b{m}",
                                name=f"psb{m}") for m in range(4)]
                kb = 0
                for s in range(T):
                    scan_step(s)
                    while kb * T < (s + 1) * NKG:
                        passb_chunk(psb, kb)
                        kb += 1
                while kb < NKG:
                    passb_chunk(psb, kb)
                    kb += 1
                for m in range(4):
                    nc.vector.tensor_scalar_mul(PRED[:, m, :], psb[m][:],
                                                1.0 / 64.0)
                pB_cm.__exit__(None, None, None)
                psc_cm.__exit__(None, None, None)

                pmix_cm = tc.tile_pool(name="pmix", bufs=1, space="PSUM")
                pmix = pmix_cm.__enter__()

                # ---- h0 = W_attn1 @ [h_f; h_b] + b_attn1 ; q = Whh_d @ h0 + bhh_d ----
                phq = pmix.tile([128, 2, 4, 4], dt.float32, tag="phq")
                nc.vector.memset(phq[:], 0.0)
                for m in range(4):
                    for k in range(8):
                        rh = OUTS[:, 0, k, (T - 1)::T] if k < 4 else \
                            OUTS[:, 1, k - 4, (T - 1)::T]
                        nc.tensor.matmul(phq[:, 0, m, :],
                                         a1[:, k, m * 128:(m + 1) * 128], rh,
                                         start=False, stop=(k == 7),
                                         skip_group_check=True)
                for m in range(4):
                    nc.vector.tensor_scalar_add(H0[:, m, :], phq[:, 0, m, :],
                                                cst[:, m:m + 1])
                for m in range(4):
                    for j in range(4):
                        nc.tensor.matmul(
                            phq[:, 1, m, :],
                            whh[:, j, 2 * H + m * 128:2 * H + (m + 1) * 128],
                            H0[:, j, :], start=False, stop=(j == 3),
                            skip_group_check=True)
                for m in range(4):
                    nc.vector.tensor_scalar_add(Q[:, m, :], phq[:, 1, m, :],
                                                cst[:, 8 + m:9 + m])

                # ---- DEC = tanh(PRED + q) ----
                for m in range(4):
                    for b in range(BL):
                        nc.scalar.activation(
                            DEC[:, m, b * T:(b + 1) * T], PRED[:, m, b * T:(b + 1) * T],
                            AF.Tanh, bias=Q[:, m, b:b + 1],
                        )

                # DEC -> fp8 hi/lo (k-tiles 4..7 of the final contraction)
                for m in range(4):
                    nc.gpsimd.tensor_copy(CAT8H[:, 4 + m, :], DEC[:, m, :])
                for m in range(4):
                    nc.gpsimd.tensor_sub(CAT8L[:, 4 + m, :], DEC[:, m, :],
                                         CAT8H[:, 4 + m, :])

                # ---- per-batch ENC/ENCT and attention, software-pipelined ----
                def enc_b(b):
                    cs = slice(b * T, (b + 1) * T)
                    pet = pmix.tile([128, H], dt.float32, tag="pet", bufs=2)
                    for k in range(8):
                        src = OUTS[:, 0, k, cs] if k < 4 else OUTB[:, k - 4, cs]
                        nc.tensor.matmul(pet[:], src, a2[:, k, :],
                                         start=(k == 0), stop=False)
                    nc.tensor.matmul(pet[:], ones[0:1, :], ba2[0:1, :],
                                     start=False, stop=True)
                    nc.vector.tensor_copy(ENCT[:, b, :], pet[:])

                wts = [None] * BL

                def scores_b(b):
                    cs = slice(b * T, (b + 1) * T)
                    ps = pmix.tile([128, T], dt.float32, tag="ps", name="ps")
                    for k in range(4):
                        nc.tensor.matmul(
                            ps[:], DEC[:, k, cs], ENC[:, k, cs],
                            start=(k == 0), stop=(k == 3),
                        )
                    negm = osp.tile([128, 1], dt.float32, tag="negm")
                    nc.vector.reduce_max(negm[:], ps[:], axis=AX.X, negate=True)
                    prob = osp.tile([128, T], dt.bfloat16, tag="prob")
                    rsum = osp.tile([128, 1], dt.float32, tag="rsum")
                    nc.scalar.activation(prob[:], ps[:], AF.Exp, bias=negm[:],
                                         accum_out=rsum[:])
                    rinv = osp.tile([128, 1], dt.float32, tag="rinv")
                    nc.vector.reciprocal(rinv[:], rsum[:])
                    nc.vector.tensor_scalar_mul(prob[:], prob[:], rinv[:])
                    wts[b] = prob

                def ctx_b(b):
                    cs = slice(b * T, (b + 1) * T)
                    pwt = pmix.tile([128, T], dt.bfloat16, tag="pwt", name="pwt")
                    nc.tensor.transpose(pwt[:], wts[b][:], idn[:])
                    wt = osp.tile([128, T], dt.bfloat16, tag="wt")
                    nc.vector.tensor_copy(wt[:], pwt[:])
                    for m in range(4):
                        pc = pmix.tile([128, T], dt.float32, tag="pc", name="pc")
                        nc.tensor.matmul(pc[:], ENCT[:, b, m * 128:(m + 1) * 128],
                                         wt[:], start=True, stop=True)
                        nc.vector.tensor_copy(CTX[:, m, cs], pc[:])

                enc_b(0)
                enc_b(1)
                scores_b(0)
                enc_b(2)
                ctx_b(0)
                scores_b(1)
                enc_b(3)
                ctx_b(1)
                scores_b(2)
                ctx_b(2)
                scores_b(3)
                ctx_b(3)

                # CTX -> fp8 hi/lo (k-tiles 0..3)
                for m in range(4):
                    nc.gpsimd.tensor_copy(CAT8H[:, m, :], CTX[:, m, :])
                for m in range(4):
                    nc.gpsimd.tensor_sub(CAT8L[:, m, :], CTX[:, m, :],
                                         CAT8H[:, m, :])

                pmix_cm.__exit__(None, None, None)

                # ---- final projection: out[v, c] = W_out.T.T @ [CTX; DEC] ----
                pf_cm = tc.tile_pool(name="pf", bufs=4, space="PSUM")
                pf = pf_cm.__enter__()
                for n in range(VT2):
                    won = ws.tile([128, 8, 512], F8, tag="won", bufs=6)
                    nc.sync.dma_start(
                        won[:], WOT8[n].rearrange("(k p) j -> p k j", p=128))
                    ov = osp.tile([128, 2, C], dt.bfloat16, tag="ov", bufs=2)
                    for u in range(2):
                        po = pf.tile([128, C], dt.float32, tag="po")
                        hs = slice(u * 128, (u + 1) * 128)
                        ls = slice(256 + u * 128, 256 + (u + 1) * 128)
                        for pr in range(4):
                            ks2 = slice(2 * pr, 2 * pr + 2)
                            terms = [(won[:, ks2, hs], CAT8H, pr == 0, False),
                                     (won[:, ks2, hs], CAT8L, False, False),
                                     (won[:, ks2, ls], CAT8H, False, pr == 3)]
                            for wsl, cat, st, sp in terms:
                                nc.tensor.matmul(
                                    po[:], wsl, cat[:, ks2, :],
                                    start=st, stop=sp, perf_mode=DR,
                                )
                        nc.scalar.activation(ov[:, u, :], po[:], AF.Identity,
                                             bias=boutp[:, 2 * n + u:2 * n + u + 1],
                                             scale=1.0 / 64.0)
                    nc.sync.dma_start(outr[:, 2 * n:2 * n + 2, :], ov[:])
                pf_cm.__exit__(None, None, None)

    nc.compile()
    return nc


def _pack(inputs):
    """Host-side packing: shared weights + per-core activation shards."""
    import ml_dtypes
    bf16 = ml_dtypes.bfloat16
    f8 = ml_dtypes.float8_e4m3
    f = {k: np.asarray(v, dtype=np.float32) for k, v in inputs.items()}

    def bf(a):
        return np.ascontiguousarray(a.astype(bf16))

    def f8pair(a):
        hi = a.astype(f8)
        lo = (a - hi.astype(np.float32)).astype(f8)
        return np.ascontiguousarray(hi), np.ascontiguousarray(lo)

    WIH = np.zeros((VP, 3 * H), np.float32)
    WIH[:V, 0:H] = f["Wih_f"].T
    WIH[:V, H:2 * H] = f["Wih_b"].T
    WIH[:V, 2 * H:] = f["Wih_d"].T
    WIH[V, 0:H] = f["bih_f"] + f["bhh_f"]
    WIH[V, H:2 * H] = f["bih_b"] + f["bhh_b"]
    WIH[V, 2 * H:] = f["bih_d"]
    WIHH, WIHL = f8pair(64.0 * WIH)

    WHH = np.concatenate([f["Whh_f"].T, f["Whh_b"].T, f["Whh_d"].T], axis=1)
    CONST = np.concatenate(
        [f["b_attn1"].reshape(4, 128).T, f["b_attn2"].reshape(4, 128).T,
         f["bhh_d"].reshape(4, 128).T], axis=1).astype(np.float32)

    WOTf = np.zeros((2 * H, VPO), np.float32)
    WOTf[:, :V] = f["W_out"].T
    WOTt = np.ascontiguousarray(
        WOTf.reshape(2 * H, VT2, 256).transpose(1, 0, 2))
    WOH, WOL = f8pair(64.0 * WOTt)
    WOT8 = np.ascontiguousarray(np.concatenate([WOH, WOL], axis=2))
    BOUTP = np.zeros(VPO, np.float32)
    BOUTP[:V] = f["b_out"]
    BOUTP = np.ascontiguousarray(BOUTP.reshape(2 * VT2, 128).T)

    shared = {
        "WIHH": WIHH,
        "WIHL": WIHL,
        "WOT8": WOT8,
        "WHH": bf(WHH),
        "A1": bf(f["W_attn1"].T),
        "A2": bf(f["W_attn2"].T),
        "CONST": np.ascontiguousarray(CONST),
        "BOUTP": BOUTP,
        "BA2": bf(f["b_attn2"].reshape(1, H)),
        "ONES": bf(np.ones((1, 128), np.float32)),
        "IDN": bf(np.eye(128, dtype=np.float32)),
    }

    # activations: [V, B*T] with column b*T + t; pad to VP with ones row at V
    def actT(a):  # [B, T, V] -> [VP, B*T] float32
        r = np.zeros((VP, B * T), np.float32)
        r[:V] = a.transpose(2, 0, 1).reshape(V, B * T)
        r[V] = 1.0
        return r

    XTH, XTL = f8pair(actT(f["enc_inputs"]))
    DXTH, DXTL = f8pair(actT(f["dec_inputs"]))

    in_maps = []
    for core in range(NCORES):
        sl = slice(core * C, (core + 1) * C)
        eh = np.zeros((128, 32), np.float32)
        for d in range(2):
            hh = f["enc_hidden"][d, core * BL:(core + 1) * BL]     # [4, 512]
            eh[:, d * 16:(d + 1) * 16] = \
                hh.T.reshape(4, 128, 4).transpose(1, 0, 2).reshape(128, 16)
        m = dict(shared)
        m["xTH"] = np.ascontiguousarray(XTH[:, sl])
        m["xTL"] = np.ascontiguousarray(XTL[:, sl])
        m["dxTH"] = np.ascontiguousarray(DXTH[:, sl])
        m["dxTL"] = np.ascontiguousarray(DXTL[:, sl])
        m["ENCH"] = bf(eh)
        in_maps.append(m)
    return in_maps


def _unpack_core(res, core):
    """res: dict of this core's output tensors -> [BL, T, V] float32."""
    o = np.asarray(res["out"], dtype=np.float32)   # [VPO, C]
    return o[:V].reshape(V, BL, T).transpose(1, 2, 0)


def kernel(**inputs):
    from concourse.bass_utils import run_bass_kernel_spmd

    if "nc" not in _cached:
        _cached["nc"] = _build_nc()
    nc = _cached["nc"]
    in_maps = _pack(inputs)
    res = run_bass_kernel_spmd(
        nc, in_maps, core_ids=list(range(NCORES)),
        trace=bool(int(os.environ.get("KTRACE", "0"))),
    )
    _cached["last"] = res
    outp = np.zeros((B, T, V), np.float32)
    for core in range(NCORES):
        outp[core * BL:(core + 1) * BL] = _unpack_core(res.results[core], core)
    return outp
